# revision 9
# baseline (speedup 1.0000x reference)
"""Trainium2 Bass kernel for nn_Attention_48876727828718.

RBF-kernel causal attention with per-head full-rank projections:
  xn = LayerNorm(x) * ln_w
  Q/K/V = xn @ W_{q,k,v}[h]          (per head, [S,E]@[E,E])
  scores = exp(-gamma_h * ||q_i - k_j||^2 / sqrt(E)) * causal
  out = (scores @ V  concat heads) @ W_o.T

Sharding: B(2) x H(8) = 16 (b,h) pairs over 8 cores; core c handles
batch b = c//4 and heads {2*(c%4), 2*(c%4)+1}.  Host sums the 4 partial
outputs per batch (W_o is folded into V on device via Wvo = W_v @ W_o_blk^T).

Device algorithm per (b, h) — flash-style, scores never touch HBM.
Key factorization vs the augmented-operand variant: with
  z_jq = gs*(K_j.Q_q) - gs*k2_j/2   (gs = 2*gamma/sqrt(E))
  scores_jq = exp(z_jq) * aq_q,  aq_q = exp(-gs*q2_q/2)  (host-computed)
the contraction K drops to exactly 64, so the two heads' score matmuls
row-pack into the 128-row PE array (rows 0:63 = h0, 64:127 = h1) and run
CONCURRENTLY; the -gs*k2/2 term rides the ACT bias AP (per-partition) and
aq is a per-column scale applied once on the OT accumulator at pass end.

  - QT/KT [128, S] bf16 stacked tiles via col-packed projection matmuls
    (h0 -> psum rows 0:63, h1 -> 64:127, also concurrent)
  - T_psum[j, q] = K.Q per (h, jb) via row-packed bf16 matmuls
  - texp = exp(gs*T + bias) via ACT (scale/bias per-partition APs), bf16
  - causal mask via gpsimd affine_select on the diagonal 128-col slice
  - OT[e, q] += VW_jb^T @ texp accumulated over (h, jb) in a col-packed
    [128, 512] psum per q-super (h0 rows 0:63, h1 64:127); single
    has_written accumulation group per tile (start only on the very first
    matmul -- the bank-wide bit-clear must happen exactly once)
  - pass end: OUT[e,q] = sum_h aq[h,q] * OT[64h:64h+64, q] via two DVE
    tensor_tensor ops (mult with replicated AQ tile, then halves-add)
  - two q-super passes (supers {2,3} then {0,1}) as before
"""

import math

import numpy as np

B, S, E, H = 2, 2048, 64, 8
EPS = 1e-5
NCORES = 8
NB = S // 128  # 16 j blocks
NQ = S // 512  # 4 q supers

_BUILT = {}


def _build():
    """Build + compile the single-core Bass program (same NEFF for all cores)."""
    from contextlib import ExitStack

    import concourse.bass as bass
    import concourse.mybir as mybir
    import concourse.tile as tile
    from concourse import bacc

    fp32 = mybir.dt.float32
    f32r = mybir.dt.float32r
    bf16 = mybir.dt.bfloat16
    Exp = mybir.ActivationFunctionType.Exp
    Copy = mybir.ActivationFunctionType.Copy
    add = mybir.AluOpType.add
    mult = mybir.AluOpType.mult
    is_ge = mybir.AluOpType.is_ge

    def mm(ap):
        return ap.bitcast(f32r)

    rr = mm  # writers of f32r matmul-feeding tiles must emit rounded values

    nc = bacc.Bacc("TRN2", target_bir_lowering=False, debug=False)

    xnt_d = nc.dram_tensor("xnt", [E, S], fp32, kind="ExternalInput").ap()
    xntb_d = nc.dram_tensor("xntb", [E, S], bf16, kind="ExternalInput").ap()
    wq_d = nc.dram_tensor("wq", [2, E, E], fp32, kind="ExternalInput").ap()
    wk_d = nc.dram_tensor("wk", [2, E, E], fp32, kind="ExternalInput").ap()
    wvob_d = nc.dram_tensor("wvob", [2, E, E], bf16, kind="ExternalInput").ap()
    gsc_d = nc.dram_tensor("gsc", [2, 128], fp32, kind="ExternalInput").ap()
    bk2_d = nc.dram_tensor("bk2", [32, 128], fp32, kind="ExternalInput").ap()
    aqr_d = nc.dram_tensor("aqr", [128, S], fp32, kind="ExternalInput").ap()
    # both heads' 64-row halves; host adds them (DVE cannot cross partitions)
    out_d = nc.dram_tensor("out", [128, S], fp32, kind="ExternalOutput").ap()

    with ExitStack() as ctx:
        tc = ctx.enter_context(tile.TileContext(nc))
        const = ctx.enter_context(tc.tile_pool(name="const", bufs=1))
        sb = ctx.enter_context(tc.tile_pool(name="sb", bufs=1))
        hb = ctx.enter_context(tc.tile_pool(name="hb", bufs=1))
        texp_pool = ctx.enter_context(tc.tile_pool(name="texp", bufs=6))
        ps_T = ctx.enter_context(tc.tile_pool(name="psT", bufs=3, space="PSUM"))
        ps_ot = ctx.enter_context(tc.tile_pool(name="psot", bufs=2, space="PSUM"))

        # ---- constants ----
        zero_col = const.tile([128, 1], fp32)
        nc.gpsimd.memset(zero_col, 0.0)
        eps_col = const.tile([128, 1], fp32)
        nc.gpsimd.memset(eps_col, EPS)
        nc.const_aps.aps[(fp32, 0.0)] = zero_col
        nc.const_aps.aps[(fp32, EPS)] = eps_col

        # ---- inputs ----
        xnT = sb.tile([E, S], fp32)
        nc.sync.dma_start(rr(xnT), rr(xnt_d))
        xnTb = sb.tile([E, S], bf16)
        nc.sync.dma_start(xnTb, xntb_d)
        # proj weights: dest[e, h, f] = W[h, e, f]; stage then round to f32r
        wq_st = const.tile([E, 2 * E], fp32)
        nc.sync.dma_start(
            wq_st.rearrange("e (h f) -> e h f", h=2), wq_d.transpose([1, 0, 2])
        )
        wk_st = const.tile([E, 2 * E], fp32)
        nc.sync.dma_start(
            wk_st.rearrange("e (h f) -> e h f", h=2), wk_d.transpose([1, 0, 2])
        )
        wvo_sb = const.tile([E, 2 * E], bf16)
        nc.sync.dma_start(
            wvo_sb.rearrange("e (h f) -> e h f", h=2), wvob_d.transpose([1, 0, 2])
        )
        wq_sb = const.tile([E, 2 * E], fp32)
        nc.vector.tensor_copy(rr(wq_sb), wq_st)
        wk_sb = const.tile([E, 2 * E], fp32)
        nc.vector.tensor_copy(rr(wk_sb), wk_st)
        gsc_sb = const.tile([128, 2], fp32)
        nc.sync.dma_start(gsc_sb, gsc_d.transpose([1, 0]))
        bk2_sb = const.tile([128, 32], fp32)
        nc.sync.dma_start(bk2_sb, bk2_d.transpose([1, 0]))
        AQ = sb.tile([128, S], fp32)
        nc.sync.dma_start(AQ, aqr_d)

        OUTsb = sb.tile([128, S], fp32)

        # ---- projections: QT/KT stacked [128, S] bf16 (h0 rows 0:63, h1
        # rows 64:127).  Col-packed matmul pairs write one [128, 512] psum
        # (concurrent on the PE); one full-height copy evacuates both heads.
        QTs = hb.tile([128, S], bf16, name="QTs", tag="qts")
        KTs = hb.tile([128, S], bf16, name="KTs", tag="kts")
        # order: all KT supers, then QT supers 2,3 (needed first by pass
        # sp0=2), then QT 0,1 (hidden behind the first pass)
        proj_work = [(KTs, wk_sb, c4) for c4 in range(NQ)]
        proj_work += [(QTs, wq_sb, c4) for c4 in (2, 3, 0, 1)]
        for idx, (dst, w_sb, c4) in enumerate(proj_work):
            pp = ps_ot.tile([128, 512], fp32, name=f"pp{idx}", tag="ot")
            # one matmul projects BOTH heads: lhsT [64, 128] has h0 in cols
            # 0:63 and h1 in cols 64:127 -> out rows are the stacked heads
            nc.tensor.matmul(
                pp,
                mm(w_sb),
                mm(xnT[:, c4 * 512 : (c4 + 1) * 512]),
                start=True,
                stop=True,
            )
            dslice = dst[:, c4 * 512 : (c4 + 1) * 512]
            if idx % 2 == 0:
                nc.scalar.activation(dslice, pp, Copy)
            else:
                nc.vector.tensor_copy(dslice, pp)

        # ---- VW = xn @ (W_v @ W_o_blk^T) per head, bf16 rows layout ----
        VWs = {}
        for h in range(2):
            VW = hb.tile([128, NB * E], bf16, name=f"VW{h}", tag=f"vw{h}")
            VWs[h] = VW
            for g in range(4):
                pv = ps_ot.tile([128, 256], fp32, name=f"pv{h}{g}", tag="ot")
                for k in range(4):
                    jb = 4 * g + k
                    nc.tensor.matmul(
                        pv[:, k * E : (k + 1) * E],
                        xnTb[:, jb * 128 : (jb + 1) * 128],
                        wvo_sb[:, h * E : (h + 1) * E],
                        start=True,
                        stop=True,
                    )
                if (h + g) % 2 == 0:
                    nc.vector.tensor_copy(VW[:, g * 256 : (g + 1) * 256], pv)
                else:
                    nc.scalar.activation(VW[:, g * 256 : (g + 1) * 256], pv, Copy)

        # ---- main loop: two q-super passes; j-blocks outer, heads
        # row-packed on the PE; both heads accumulate into the same
        # col-packed OT psum ----
        for sp0 in (2, 0):
            OTp = [
                ps_ot.tile([128, 512], fp32, name=f"ot{sp0}{i}", tag="ot")
                for i in range(2)
            ]
            # zero the accumulator banks and use start=False on every OT
            # matmul: the two heads' col-packed regions share a bank, and a
            # start=True bank-wide bit-clear from one head can race the
            # other head's accumulation (no data dep orders them).  With
            # zeroed data, accumulate-onto-stale-bits == overwrite == 0+x.
            for i in range(2):
                nc.vector.memset(OTp[i], 0.0)
            jb_max = 8 if sp0 == 0 else NB

            def emit_ot(args, sp0=sp0, OTp=OTp):
                texp_, VW_, jb_, qs_first_, w_, dead_, h_ = args
                for s5 in range(w_ // 512):
                    qs = qs_first_ + s5
                    n0 = dead_ if s5 == 0 else 0
                    tlo = s5 * 512 + n0 - dead_
                    nc.tensor.matmul(
                        OTp[qs - sp0][h_ * E : (h_ + 1) * E, n0:512],
                        VW_[:, jb_ * E : (jb_ + 1) * E],
                        texp_[:, tlo : tlo + 512 - n0],
                        start=False,
                        stop=(jb_ == 4 * qs + 3 and h_ == 1),
                    )

            # software pipeline: emit T/exp of chunk k+1 before OT of chunk
            # k, so the in-order PE stream never blocks on chunk k's exp
            pend = None
            for jb in range(jb_max):
                for h in range(2):
                    qs_first = max(sp0, jb // 4)
                    qstart = 512 * qs_first
                    w = 512 * (sp0 + 2) - qstart  # 512 or 1024
                    has_diag = (jb // 4) >= sp0
                    dead = 128 * (jb % 4) if has_diag else 0
                    p0 = h * 64
                    tchunk = ps_T.tile([128, w], fp32, name=f"t{sp0}{h}{jb}", tag="T")
                    for s5 in range(w // 512):
                        n0 = dead if s5 == 0 else 0
                        q0 = qstart + s5 * 512
                        nc.tensor.matmul(
                            tchunk[:, s5 * 512 + n0 : (s5 + 1) * 512],
                            KTs[p0 : p0 + 64, jb * 128 : (jb + 1) * 128],
                            QTs[p0 : p0 + 64, q0 + n0 : q0 + 512],
                            start=True,
                            stop=True,
                        )
                    texp = texp_pool.tile(
                        [128, w - dead], bf16, name=f"te{sp0}{h}{jb}", tag="te"
                    )
                    nc.scalar.activation(
                        texp,
                        tchunk[:, dead:w],
                        Exp,
                        bias=bk2_sb[:, h * 16 + jb : h * 16 + jb + 1],
                        scale=gsc_sb[:, h : h + 1],
                    )
                    if has_diag:
                        # causal mask: texp col c is global q = 128*jb + c
                        # vs j = 128*jb + p, keep c >= p
                        nc.gpsimd.affine_select(
                            out=texp[:, 0:128],
                            in_=texp[:, 0:128],
                            pattern=[[1, 128]],
                            compare_op=is_ge,
                            fill=0.0,
                            base=0,
                            channel_multiplier=-1,
                        )
                    if pend is not None:
                        emit_ot(pend)
                    pend = (texp, VWs[h], jb, qs_first, w, dead, h)
            emit_ot(pend)
            for i in range(2):
                qs = sp0 + i
                qsl = slice(qs * 512, (qs + 1) * 512)
                nc.vector.tensor_tensor(OUTsb[:, qsl], OTp[i], AQ[:, qsl], op=mult)
                nc.sync.dma_start(out_d[:, qsl], OUTsb[:, qsl])

    nc.compile()
    return nc


def _get_nc():
    if 0 not in _BUILT:
        _BUILT[0] = _build()
    return _BUILT[0]


def _prep_inputs(x, ln_w, W_q, W_k, W_v, W_o, gamma):
    """Host-side input prep: fold weights, stat scales, shard per core."""
    import ml_dtypes

    x = np.asarray(x, np.float32)
    ln_w = np.asarray(ln_w, np.float32)
    W_q = np.asarray(W_q, np.float32)
    W_k = np.asarray(W_k, np.float32)
    W_v = np.asarray(W_v, np.float32)
    W_o = np.asarray(W_o, np.float32)
    gamma = np.asarray(gamma, np.float32).reshape(H)

    # fold ln_w into projection weights; fold W_o into W_v
    lw = ln_w[None, :, None]  # [1, E, 1] scale on contraction dim e
    Wq = (W_q * lw).astype(np.float32)
    Wk = (W_k * lw).astype(np.float32)
    Wv = (W_v * lw).astype(np.float32)
    Wo_blk = W_o.reshape(E, H, E).transpose(1, 0, 2)  # [H, e_out, f]
    Wvo = np.einsum("hef,hof->heo", Wv.astype(np.float64), Wo_blk.astype(np.float64))
    Wvo = Wvo.astype(np.float32)  # [H, e, e_out]
    gs = (2.0 * gamma / np.sqrt(E)).astype(np.float32)  # exp scale per head

    # host-computed per-(b,h) stats: k2 -> ACT bias rows, q2 -> aq column
    # scale applied on the OT accumulator
    mu = x.mean(-1, keepdims=True)
    var = ((x - mu) ** 2).mean(-1, keepdims=True)
    xn = (x - mu) / np.sqrt(var + EPS)  # ln_w folded into weights
    Qh = np.einsum("bse,hef->bhsf", xn, Wq)  # [B,H,S,E]
    Kh = np.einsum("bse,hef->bhsf", xn, Wk)
    q2 = (Qh * Qh).sum(-1)  # [B,H,S]
    k2 = (Kh * Kh).sum(-1)

    in_maps = []
    for c in range(NCORES):
        b = c // 4
        h0 = 2 * (c % 4)
        xnt = np.ascontiguousarray(xn[b].T.astype(np.float32))
        # bk2[h*16+jb, p] = -gs_h * k2[b, h0+h, jb*128+p] / 2
        bk2 = np.zeros((32, 128), np.float32)
        aqr = np.zeros((128, S), np.float32)
        for h in range(2):
            bk2[h * 16 : (h + 1) * 16, :] = (
                -0.5 * gs[h0 + h] * k2[b, h0 + h]
            ).reshape(16, 128)
            aqr[h * 64 : (h + 1) * 64, :] = np.exp(
                -0.5 * gs[h0 + h] * q2[b, h0 + h]
            )[None, :]
        in_maps.append(
            {
                "xnt": xnt,
                "xntb": xnt.astype(ml_dtypes.bfloat16),
                "wq": np.ascontiguousarray(Wq[h0 : h0 + 2]),
                "wk": np.ascontiguousarray(Wk[h0 : h0 + 2]),
                "wvob": np.ascontiguousarray(
                    Wvo[h0 : h0 + 2].astype(ml_dtypes.bfloat16)
                ),
                "gsc": np.ascontiguousarray(
                    np.broadcast_to(gs[h0 : h0 + 2, None], (2, 128))
                ),
                "bk2": bk2,
                "aqr": aqr,
            }
        )
    return in_maps


def kernel(x, ln_w, W_q, W_k, W_v, W_o, gamma):
    from concourse import bass_utils

    nc = _get_nc()
    in_maps = _prep_inputs(x, ln_w, W_q, W_k, W_v, W_o, gamma)
    res = bass_utils.run_bass_kernel_spmd(nc, in_maps, core_ids=list(range(NCORES)))

    out = np.zeros((B, S, E), np.float32)
    for c in range(NCORES):
        o = res.results[c]["out"]  # [128, S]: rows 0:64 = h0, 64:128 = h1
        out[c // 4] += o[0:64].T + o[64:128].T
    return out


# revision 11
# speedup vs baseline: 1.0542x; 1.0542x over previous
"""Trainium2 Bass kernel for nn_Attention_48876727828718.

RBF-kernel causal attention with per-head full-rank projections:
  xn = LayerNorm(x) * ln_w
  Q/K/V = xn @ W_{q,k,v}[h]          (per head, [S,E]@[E,E])
  scores = exp(-gamma_h * ||q_i - k_j||^2 / sqrt(E)) * causal
  out = (scores @ V  concat heads) @ W_o.T

Sharding: B(2) x H(8) = 16 (b,h) pairs over 8 cores; core c handles
batch b = c//4 and heads {2*(c%4), 2*(c%4)+1}.  Host sums the per-head
output halves and the 4 partial outputs per batch.

Factorized device algorithm per (b, h), flash-style (scores never touch
HBM).  With gs = 2*gamma/sqrt(E):
  scores_jq = exp(gs*K_j.Q_q) * bfac_j * aq_q
  bfac_j = exp(-gs*k2_j/2)   -> folded into VW on host (xnb = xn*bfac)
  aq_q   = exp(-gs*q2_q/2)   -> per-column scale on the OT accumulator
  gs     -> folded into W_q on host, so the device exp has NO scale/bias
The contraction K is exactly 64, so the two heads row-pack into the
128-row PE array (h0 rows 0:63, h1 64:127) and their score matmuls run
concurrently; the OT matmuls col-pack (h0 -> psum rows 0:63, h1 ->
64:127).  Pairs are emitted adjacently so the PE can overlap them.
"""

import math

import numpy as np

B, S, E, H = 2, 2048, 64, 8
EPS = 1e-5
NCORES = 8
NB = S // 128  # 16 j blocks
NQ = S // 512  # 4 q supers

_BUILT = {}


def _build():
    """Build + compile the single-core Bass program (same NEFF for all cores)."""
    from contextlib import ExitStack

    import concourse.bass as bass
    import concourse.mybir as mybir
    import concourse.tile as tile
    from concourse import bacc

    fp32 = mybir.dt.float32
    f32r = mybir.dt.float32r
    bf16 = mybir.dt.bfloat16
    Exp = mybir.ActivationFunctionType.Exp
    Copy = mybir.ActivationFunctionType.Copy
    mult = mybir.AluOpType.mult
    is_ge = mybir.AluOpType.is_ge

    def mm(ap):
        return ap.bitcast(f32r)

    rr = mm  # writers of f32r matmul-feeding tiles must emit rounded values

    nc = bacc.Bacc("TRN2", target_bir_lowering=False, debug=False)

    xnt_d = nc.dram_tensor("xnt", [E, S], fp32, kind="ExternalInput").ap()
    xnb_d = nc.dram_tensor("xnb", [2, E, S], bf16, kind="ExternalInput").ap()
    wq_d = nc.dram_tensor("wq", [2, E, E], fp32, kind="ExternalInput").ap()
    wk_d = nc.dram_tensor("wk", [2, E, E], fp32, kind="ExternalInput").ap()
    wvob_d = nc.dram_tensor("wvob", [2, E, E], bf16, kind="ExternalInput").ap()
    aqr_d = nc.dram_tensor("aqr", [128, S], fp32, kind="ExternalInput").ap()
    # both heads' 64-row halves; host adds them (DVE cannot cross partitions)
    out_d = nc.dram_tensor("out", [128, S], fp32, kind="ExternalOutput").ap()

    with ExitStack() as ctx:
        tc = ctx.enter_context(tile.TileContext(nc))
        const = ctx.enter_context(tc.tile_pool(name="const", bufs=1))
        sb = ctx.enter_context(tc.tile_pool(name="sb", bufs=1))
        hb = ctx.enter_context(tc.tile_pool(name="hb", bufs=1))
        texp_pool = ctx.enter_context(tc.tile_pool(name="texp", bufs=6))
        ps_T = ctx.enter_context(tc.tile_pool(name="psT", bufs=3, space="PSUM"))
        ps_ot = ctx.enter_context(tc.tile_pool(name="psot", bufs=2, space="PSUM"))

        # ---- constants ----
        zero_col = const.tile([128, 1], fp32)
        nc.gpsimd.memset(zero_col, 0.0)
        nc.const_aps.aps[(fp32, 0.0)] = zero_col

        # ---- inputs ----
        xnT = sb.tile([E, S], fp32)
        nc.sync.dma_start(rr(xnT), rr(xnt_d))
        xnb = {}
        for h in range(2):
            xnb[h] = sb.tile([E, S], bf16, name=f"xnb{h}")
            nc.sync.dma_start(xnb[h], xnb_d[h])
        # proj weights: dest[e, h, f] = W[h, e, f]; stage then round to f32r
        wq_st = const.tile([E, 2 * E], fp32)
        nc.sync.dma_start(
            wq_st.rearrange("e (h f) -> e h f", h=2), wq_d.transpose([1, 0, 2])
        )
        wk_st = const.tile([E, 2 * E], fp32)
        nc.sync.dma_start(
            wk_st.rearrange("e (h f) -> e h f", h=2), wk_d.transpose([1, 0, 2])
        )
        wvo_sb = const.tile([E, 2 * E], bf16)
        nc.sync.dma_start(
            wvo_sb.rearrange("e (h f) -> e h f", h=2), wvob_d.transpose([1, 0, 2])
        )
        wq_sb = const.tile([E, 2 * E], fp32)
        nc.vector.tensor_copy(rr(wq_sb), wq_st)
        wk_sb = const.tile([E, 2 * E], fp32)
        nc.vector.tensor_copy(rr(wk_sb), wk_st)
        AQ = sb.tile([128, S], fp32)
        nc.sync.dma_start(AQ, aqr_d)

        OUTsb = sb.tile([128, S], fp32)

        # ---- projections: QT/KT stacked [128, S] bf16 (h0 rows 0:63, h1
        # rows 64:127).  One matmul projects BOTH heads: lhsT [64, 128] has
        # h0 in cols 0:63, h1 in cols 64:127.
        QTs = hb.tile([128, S], bf16, name="QTs", tag="qts")
        KTs = hb.tile([128, S], bf16, name="KTs", tag="kts")
        proj_work = [(KTs, wk_sb, c4) for c4 in range(NQ)]
        proj_work += [(QTs, wq_sb, c4) for c4 in (2, 3, 0, 1)]
        for idx, (dst, w_sb, c4) in enumerate(proj_work):
            pp = ps_ot.tile([128, 512], fp32, name=f"pp{idx}", tag="ot")
            nc.tensor.matmul(
                pp,
                mm(w_sb),
                mm(xnT[:, c4 * 512 : (c4 + 1) * 512]),
                start=True,
                stop=True,
            )
            dslice = dst[:, c4 * 512 : (c4 + 1) * 512]
            if idx % 2 == 0:
                nc.scalar.activation(dslice, pp, Copy)
            else:
                nc.vector.tensor_copy(dslice, pp)

        # ---- VW' = (xn * bfac) @ (W_v @ W_o_blk^T) per head, bf16 ----
        VWs = {}
        for h in range(2):
            VW = hb.tile([128, NB * E], bf16, name=f"VW{h}", tag=f"vw{h}")
            VWs[h] = VW
            for g in range(4):
                pv = ps_ot.tile([128, 256], fp32, name=f"pv{h}{g}", tag="ot")
                for k in range(4):
                    jb = 4 * g + k
                    nc.tensor.matmul(
                        pv[:, k * E : (k + 1) * E],
                        xnb[h][:, jb * 128 : (jb + 1) * 128],
                        wvo_sb[:, h * E : (h + 1) * E],
                        start=True,
                        stop=True,
                    )
                if (h + g) % 2 == 0:
                    nc.vector.tensor_copy(VW[:, g * 256 : (g + 1) * 256], pv)
                else:
                    nc.scalar.activation(VW[:, g * 256 : (g + 1) * 256], pv, Copy)

        # ---- main loop: two q-super passes; per jb both heads' T matmuls
        # are emitted adjacently (row-packed, concurrent), then both exps,
        # then both pending OT chunks (col-packed pairs adjacent) ----
        for sp0 in (2, 0):
            OTp = [
                ps_ot.tile([128, 512], fp32, name=f"ot{sp0}{i}", tag="ot")
                for i in range(2)
            ]
            # zero the accumulators; all OT matmuls use start=False so the
            # two heads' col-packed regions never race on the bank-wide
            # has_written clear (accumulate-onto-zero == overwrite).
            for i in range(2):
                nc.vector.memset(OTp[i], 0.0)
            jb_max = 8 if sp0 == 0 else NB

            def emit_ot(pend2, sp0=sp0, OTp=OTp):
                # emit the two heads' OT matmuls interleaved per q-chunk so
                # the col-packed pairs sit adjacent in the PE queue
                by_s5 = {}
                for texp_, VW_, jb_, qs_first_, w_, dead_, h_ in pend2:
                    for s5 in range(w_ // 512):
                        by_s5.setdefault(s5, []).append(
                            (texp_, VW_, jb_, qs_first_, dead_, h_)
                        )
                for s5, items in sorted(by_s5.items()):
                    for texp_, VW_, jb_, qs_first_, dead_, h_ in items:
                        qs = qs_first_ + s5
                        n0 = dead_ if s5 == 0 else 0
                        tlo = s5 * 512 + n0 - dead_
                        nc.tensor.matmul(
                            OTp[qs - sp0][h_ * E : (h_ + 1) * E, n0:512],
                            VW_[:, jb_ * E : (jb_ + 1) * E],
                            texp_[:, tlo : tlo + 512 - n0],
                            start=False,
                            stop=(jb_ == 4 * qs + 3 and h_ == 1),
                        )

            pend = None
            for jb in range(jb_max):
                qs_first = max(sp0, jb // 4)
                qstart = 512 * qs_first
                w = 512 * (sp0 + 2) - qstart  # 512 or 1024
                has_diag = (jb // 4) >= sp0
                dead = 128 * (jb % 4) if has_diag else 0
                cur = []
                tch = {}
                for h in range(2):
                    p0 = h * 64
                    tchunk = ps_T.tile([128, w], fp32, name=f"t{sp0}{h}{jb}", tag="T")
                    tch[h] = tchunk
                    for s5 in range(w // 512):
                        n0 = dead if s5 == 0 else 0
                        q0 = qstart + s5 * 512
                        nc.tensor.matmul(
                            tchunk[:, s5 * 512 + n0 : (s5 + 1) * 512],
                            KTs[p0 : p0 + 64, jb * 128 : (jb + 1) * 128],
                            QTs[p0 : p0 + 64, q0 + n0 : q0 + 512],
                            start=True,
                            stop=True,
                        )
                for h in range(2):
                    texp = texp_pool.tile(
                        [128, w - dead], bf16, name=f"te{sp0}{h}{jb}", tag="te"
                    )
                    nc.scalar.activation(texp, tch[h][:, dead:w], Exp)
                    if has_diag:
                        # causal mask: texp col c is global q = 128*jb + c
                        # vs j = 128*jb + p, keep c >= p
                        nc.gpsimd.affine_select(
                            out=texp[:, 0:128],
                            in_=texp[:, 0:128],
                            pattern=[[1, 128]],
                            compare_op=is_ge,
                            fill=0.0,
                            base=0,
                            channel_multiplier=-1,
                        )
                    cur.append((texp, VWs[h], jb, qs_first, w, dead, h))
                if pend is not None:
                    emit_ot(pend)
                pend = cur
            emit_ot(pend)
            for i in range(2):
                qs = sp0 + i
                qsl = slice(qs * 512, (qs + 1) * 512)
                nc.vector.tensor_tensor(OUTsb[:, qsl], OTp[i], AQ[:, qsl], op=mult)
                nc.sync.dma_start(out_d[:, qsl], OUTsb[:, qsl])

    nc.compile()
    return nc


def _get_nc():
    if 0 not in _BUILT:
        _BUILT[0] = _build()
    return _BUILT[0]


def _prep_inputs(x, ln_w, W_q, W_k, W_v, W_o, gamma):
    """Host-side input prep: fold weights, stat scales, shard per core."""
    import ml_dtypes

    x = np.asarray(x, np.float32)
    ln_w = np.asarray(ln_w, np.float32)
    W_q = np.asarray(W_q, np.float32)
    W_k = np.asarray(W_k, np.float32)
    W_v = np.asarray(W_v, np.float32)
    W_o = np.asarray(W_o, np.float32)
    gamma = np.asarray(gamma, np.float32).reshape(H)

    # fold ln_w into projection weights; fold W_o into W_v
    lw = ln_w[None, :, None]  # [1, E, 1] scale on contraction dim e
    Wq = (W_q * lw).astype(np.float32)
    Wk = (W_k * lw).astype(np.float32)
    Wv = (W_v * lw).astype(np.float32)
    Wo_blk = W_o.reshape(E, H, E).transpose(1, 0, 2)  # [H, e_out, f]
    Wvo = np.einsum("hef,hof->heo", Wv.astype(np.float64), Wo_blk.astype(np.float64))
    Wvo = Wvo.astype(np.float32)  # [H, e, e_out]
    gs = (2.0 * gamma / np.sqrt(E)).astype(np.float32)  # exp scale per head

    # host-computed per-(b,h) stats
    mu = x.mean(-1, keepdims=True)
    var = ((x - mu) ** 2).mean(-1, keepdims=True)
    xn = (x - mu) / np.sqrt(var + EPS)  # ln_w folded into weights
    Qh = np.einsum("bse,hef->bhsf", xn, Wq)  # [B,H,S,E]
    Kh = np.einsum("bse,hef->bhsf", xn, Wk)
    q2 = (Qh * Qh).sum(-1)  # [B,H,S]
    k2 = (Kh * Kh).sum(-1)

    in_maps = []
    for c in range(NCORES):
        b = c // 4
        h0 = 2 * (c % 4)
        xnt = np.ascontiguousarray(xn[b].T.astype(np.float32))  # [E, S]
        aqr = np.zeros((128, S), np.float32)
        xnb = np.zeros((2, E, S), np.float32)
        wqs = np.zeros((2, E, E), np.float32)
        for h in range(2):
            g = gs[h0 + h]
            aqr[h * 64 : (h + 1) * 64, :] = np.exp(-0.5 * g * q2[b, h0 + h])[None, :]
            bfac = np.exp(-0.5 * g * k2[b, h0 + h]).astype(np.float32)  # [S]
            xnb[h] = xnt * bfac[None, :]
            wqs[h] = g * Wq[h0 + h]  # gs folded into W_q
        in_maps.append(
            {
                "xnt": xnt,
                "xnb": xnb.astype(ml_dtypes.bfloat16),
                "wq": wqs,
                "wk": np.ascontiguousarray(Wk[h0 : h0 + 2]),
                "wvob": np.ascontiguousarray(
                    Wvo[h0 : h0 + 2].astype(ml_dtypes.bfloat16)
                ),
                "aqr": aqr,
            }
        )
    return in_maps


def kernel(x, ln_w, W_q, W_k, W_v, W_o, gamma):
    from concourse import bass_utils

    nc = _get_nc()
    in_maps = _prep_inputs(x, ln_w, W_q, W_k, W_v, W_o, gamma)
    res = bass_utils.run_bass_kernel_spmd(nc, in_maps, core_ids=list(range(NCORES)))

    out = np.zeros((B, S, E), np.float32)
    for c in range(NCORES):
        o = res.results[c]["out"]  # [128, S]: rows 0:64 = h0, 64:128 = h1
        out[c // 4] += o[0:64].T + o[64:128].T
    return out
